# revision 37
# baseline (speedup 1.0000x reference)
"""Trainium2 Bass kernel for nn_AttentionLayer (pre-conv + self-attention + final conv).

Sharding: 8 cores = 2 samples x 4 query-row chunks. Each core computes the
full pre-conv y for its sample (k/v need all N=9216 positions), attention for
its 26-row query window (24 own rows + 1 halo row each side for the final
3x3 conv), and the final conv for its 24 output rows.

Perf structure (v2):
 - exp is split between ScalarE (native Exp) and VectorE (Schraudolph int16
   trick: bf16 bits = round(x * 2^7/ln2 + (127*2^7 - 5.51)), written as
   uint16 and bitcast to bf16) in alternating groups of 3 j-blocks, so both
   engines chew the 23M-element softmax concurrently.
 - energy matmuls run 4-way concurrent via tile_position row groups; k is
   laid out in 4 partition stripes directly by col-tiled projection matmuls,
   q is replicated to all 4 stripes by a single 4-replica weight matrix.
 - softmax denominator comes from an augmented ones-column in vT; the
   divide is reciprocal_approx_fast on [1,NI] + gpsimd partition_broadcast;
   the residual-add + padded-layout scatter runs on gpsimd.
 - the final 3x3 conv is 2-row-tap packed (6 matmuls per 4 rows).
"""

import os
import hashlib
import shutil

import numpy as np
import ml_dtypes

BF16 = ml_dtypes.bfloat16
EPS = 1e-5

B, C, CQK, H, W = 2, 64, 16, 96, 96
N = H * W                       # 9216
QCH = 4                         # query chunks per sample
ROWS = H // QCH                 # 24 rows per core
LOCROWS = ROWS + 2              # 26 (with halo)
NLOC = LOCROWS * W              # 2496
HP, WP = H + 2, W + 2           # 98x98 padded frame
LOCP = LOCROWS + 2              # 28 padded local rows
NI_SIZES = [512, 512, 512, 512, 448]   # i-chunks over NLOC
JB = 128                        # j-block height
NJB = N // JB                   # 72
JG = 3                          # j-blocks per exp group
# Schraudolph bf16-bits exp constants (calibrated on the real energy range
# [-5.2, 4.5]; max rel err ~3.3%)
SEXP_A = 128.0 / float(np.log(2.0))
SEXP_B = 127.0 * 128.0 - 5.51


def _jb_sequence():
    """j-block processing order: rotate the 4 k-stripes every matmul."""
    seq = []
    for u in range(4):          # chunk quads 0..15
        for b in range(4):
            for s in range(4):
                seq.append(4 * (4 * u + s) + b)
    for b in range(4):          # chunks 16,17 (stripes 0,1)
        for s in range(2):
            seq.append(4 * (16 + s) + b)
    assert sorted(seq) == list(range(NJB))
    return seq


# ---------------------------------------------------------------------------
# framework patches (self-contained)
# ---------------------------------------------------------------------------

def _apply_patches():
    import concourse.tile as tile
    import concourse.bass_utils as bu
    import concourse.bass2jax as b2j
    from concourse import mybir

    # 1) walrus in this env rejects >1-2 sync waits on the final Drain
    #    (CTRL_NO_STRUCT): split waits into single-wait nops.
    def _drain_and_barrier_split(self, tick_clock, wait_clock):
        nc = self.nc
        probe = nc.sync.nop()
        wait_clock.add_sem_waits(
            probe.ins, tile.ScopedClock({None: tick_clock.global_clock})
        )
        waits = list(probe.ins.sync_info.on_wait) if probe.ins.sync_info else []
        if probe.ins.sync_info is not None:
            probe.ins.sync_info.on_wait = []
        for w in waits[:-1]:
            nop = nc.sync.nop()
            if nop.ins.sync_info is None:
                nop.ins.sync_info = mybir.SyncInfo(on_wait=[w], on_update=[])
            else:
                nop.ins.sync_info.on_wait.append(w)
        drain_inst = nc.sync.drain()
        if waits:
            if drain_inst.ins.sync_info is None:
                drain_inst.ins.sync_info = mybir.SyncInfo(
                    on_wait=[waits[-1]], on_update=[]
                )
            else:
                drain_inst.ins.sync_info.on_wait.append(waits[-1])
        nc.all_engine_barrier()
        assert self.sems is not None
        popped = nc._tile_sem_poison_stack.pop()
        assert popped is self._sem_poison
        nc.clear_and_free_semaphores(list(self.sems.allocated().values()))
        nc.all_engine_barrier()

    tile.TileContext._drain_and_barrier = _drain_and_barrier_split

    # 2) NEFF disk cache keyed by BIR hash (compile is deterministic).
    cache_dir = os.path.join(os.path.dirname(os.path.abspath(__file__)),
                             ".neff_cache")
    try:
        os.makedirs(cache_dir, exist_ok=True)
    except OSError:
        cache_dir = None
    _orig_compile = bu.compile_bir_kernel

    def cached_compile(bir_json, tmpdir, neff_name="file.neff"):
        if cache_dir is None:
            return _orig_compile(bir_json, tmpdir, neff_name)
        h = hashlib.sha256(bir_json).hexdigest()[:24]
        cpath = os.path.join(cache_dir, f"{h}.neff")
        out = os.path.join(tmpdir, neff_name)
        if os.path.exists(cpath):
            shutil.copyfile(cpath, out)
            return out
        r = _orig_compile(bir_json, tmpdir, neff_name)
        try:
            shutil.copyfile(r, cpath)
        except OSError:
            pass
        return r

    bu.compile_bir_kernel = cached_compile
    b2j.compile_bir_kernel = cached_compile


def _split_excess_waits(nc, max_waits=1):
    """walrus in this env allows only a couple of sync-wait slots per
    instruction; move excess waits onto preceding same-engine NOPs."""
    from concourse import mybir
    idx = 0
    for f in nc.m.functions:
        for bb in f.blocks:
            new = []
            changed = False
            for inst in bb.instructions:
                si = inst.sync_info
                waits = list(si.on_wait) if si is not None and si.on_wait else []
                if len(waits) > max_waits:
                    changed = True
                    for w in waits[:-max_waits]:
                        idx += 1
                        nop = mybir.InstNoOp(name=f"wsplit_{idx}", ins=[], outs=[])
                        nop.engine = inst.engine
                        nop.sync_info = mybir.SyncInfo(on_wait=[w], on_update=[])
                        new.append(nop)
                    si.on_wait = waits[-max_waits:]
                new.append(inst)
            if changed:
                bb.instructions = new


# ---------------------------------------------------------------------------
# device program
# ---------------------------------------------------------------------------

_NC_CACHE = {}


def _build_nc(split_waits=True):
    key = ("nc", split_waits)
    if key in _NC_CACHE:
        return _NC_CACHE[key]
    _apply_patches()
    import concourse.bass as bass
    import concourse.tile as tile
    from concourse import mybir
    from contextlib import ExitStack

    f32 = mybir.dt.float32
    bf16 = mybir.dt.bfloat16
    u16 = mybir.dt.uint16
    RELU = mybir.ActivationFunctionType.Relu
    EXP = mybir.ActivationFunctionType.Exp

    nc = bass.Bass()

    xf_d = nc.declare_dram_parameter("xf", [C, HP * WP], bf16, isOutput=False)
    xl_d = nc.declare_dram_parameter("xl", [C, LOCP * WP], bf16, isOutput=False)
    # pre/final conv weights: taps (dr0|dr1) stacked on 128 partitions, dr2 sep
    wpre_d = nc.declare_dram_parameter("wpre", [2 * C, 3 * C], bf16, isOutput=False)
    wpre2_d = nc.declare_dram_parameter("wpre2", [C, 3 * C], bf16, isOutput=False)
    b1_d = nc.declare_dram_parameter("b1", [C, 1], f32, isOutput=False)
    wfin_d = nc.declare_dram_parameter("wfin", [2 * C, 3 * C], bf16, isOutput=False)
    wfin2_d = nc.declare_dram_parameter("wfin2", [C, 3 * C], bf16, isOutput=False)
    b2_d = nc.declare_dram_parameter("b2", [C, 1], f32, isOutput=False)
    wq4_d = nc.declare_dram_parameter("wq4", [C + 1, 128], bf16, isOutput=False)
    wk_d = nc.declare_dram_parameter("wk", [C + 1, CQK], bf16, isOutput=False)
    wv_d = nc.declare_dram_parameter("wv", [C + 1, C + 1], bf16, isOutput=False)
    ones_d = nc.declare_dram_parameter("ones", [1, N], bf16, isOutput=False)
    m2_d = nc.declare_dram_parameter("m2", [C, 2 * W], f32, isOutput=False)
    out_d = nc.declare_dram_parameter("out", [C, ROWS * W], f32, isOutput=True)

    jbseq = _jb_sequence()

    with tile.TileContext(nc) as tc, ExitStack() as ctx:
        consts = ctx.enter_context(tc.tile_pool(name="consts", bufs=1))
        bigs = ctx.enter_context(tc.tile_pool(name="bigs", bufs=1))

        # --- constants ---
        wpre_sb = consts.tile([2 * C, 3 * C], bf16)
        wpre2_sb = consts.tile([C, 3 * C], bf16)
        wfin_sb = consts.tile([2 * C, 3 * C], bf16)
        wfin2_sb = consts.tile([C, 3 * C], bf16)
        b1_sb = consts.tile([C, 1], f32)
        b2_sb = consts.tile([C, 1], f32)
        wq4_sb = consts.tile([C + 1, 128], bf16)
        wk_sb = consts.tile([C + 1, CQK], bf16)
        wv_sb = consts.tile([C + 1, C + 1], bf16)
        m2_sb = consts.tile([C, 2 * W], f32)
        junk_sb = consts.tile([C, 1], f32)
        nc.sync.dma_start(out=b1_sb, in_=b1_d[:])
        nc.sync.dma_start(out=wpre_sb, in_=wpre_d[:])
        nc.sync.dma_start(out=wpre2_sb, in_=wpre2_d[:])
        # preload the exp table set before the first ReLU so the kernel pays
        # exactly one ACT table load, during the startup DMA window.
        nc.scalar.activation(out=junk_sb, in_=b1_sb, func=EXP)

        # --- x frames, rows dr0 on partitions 0-63 / dr1 (shift 1 row) 64-127
        xf_sb = bigs.tile([2 * C, HP * WP], bf16)
        xl_sb = bigs.tile([2 * C, LOCP * WP], bf16)
        nc.sync.dma_start(out=xl_sb[0:C, :], in_=xl_d[:])
        nc.sync.dma_start(out=xl_sb[C:2 * C, 0:(LOCP - 1) * WP],
                          in_=xl_d[:, WP:LOCP * WP])
        for r0, r1 in [(0, 26), (26, 50), (50, 74), (74, HP)]:
            nc.sync.dma_start(out=xf_sb[0:C, r0 * WP:r1 * WP],
                              in_=xf_d[:, r0 * WP:r1 * WP])
        for r0, r1 in [(0, 50), (50, HP - 1)]:
            nc.sync.dma_start(out=xf_sb[C:2 * C, r0 * WP:r1 * WP],
                              in_=xf_d[:, (r0 + 1) * WP:(r1 + 1) * WP])
        nc.sync.dma_start(out=wfin_sb, in_=wfin_d[:])
        nc.sync.dma_start(out=wfin2_sb, in_=wfin2_d[:])
        nc.sync.dma_start(out=b2_sb, in_=b2_d[:])
        nc.sync.dma_start(out=wq4_sb, in_=wq4_d[:])
        nc.sync.dma_start(out=wk_sb, in_=wk_d[:])
        nc.sync.dma_start(out=wv_sb, in_=wv_d[:])
        nc.sync.dma_start(out=m2_sb, in_=m2_d[:])

        xf3 = xf_sb.rearrange("p (r c) -> p r c", c=WP)
        xl3 = xl_sb.rearrange("p (r c) -> p r c", c=WP)

        ya_sb = bigs.tile([C + 1, N], bf16)       # y_aug (full sample)
        yla_sb = bigs.tile([C + 1, NLOC], bf16)   # y_aug (local window)
        k_sb = bigs.tile([128, 2560], bf16)       # k in 4 partition stripes
        q_sb = bigs.tile([128, NLOC], bf16)       # q replicated at 4 stripes
        vt_sb = bigs.tile([128, NJB * (C + 1)], bf16)
        ofp_sb = bigs.tile([2 * C, LOCROWS * WP], bf16)  # padded out_feat,
        #   partitions 64-127 = shift-1-row copy for the packed final conv
        out_sb = bigs.tile([C, ROWS * W], f32)

        nc.sync.dma_start(out=ya_sb[C:C + 1, :], in_=ones_d[:])
        nc.sync.dma_start(out=yla_sb[C:C + 1, :], in_=ones_d[:, 0:NLOC])
        # ones column of vT (softmax denominator maker)
        vt3 = vt_sb.rearrange("p (j c) -> p j c", c=C + 1)
        nc.vector.memset(vt3[:, :, C:C + 1], 1.0)
        # zero the pad columns of the padded out_feat layout
        ofp3 = ofp_sb.rearrange("p (r c) -> p r c", c=WP)
        nc.vector.memset(ofp3[:, :, 0:1], 0.0)
        nc.vector.memset(ofp3[:, :, WP - 1:WP], 0.0)

        def conv6(ps, x3, r, nr, wp_sb, wp2_sb, stop_dr2):
            """6-matmul 3x3 conv chunk: rows r..r+nr of the padded frame."""
            for ds in range(3):
                nc.tensor.matmul(
                    ps[:, :nr * W],
                    wp_sb[:, ds * C:(ds + 1) * C],
                    x3[:, r:r + nr, ds:ds + W],
                    start=(ds == 0), stop=False,
                )
            for ds in range(3):
                nc.tensor.matmul(
                    ps[:, :nr * W],
                    wp2_sb[:, ds * C:(ds + 1) * C],
                    x3[0:C, r + 2:r + 2 + nr, ds:ds + W],
                    start=False, stop=(stop_dr2 and ds == 2),
                )

        with tc.tile_pool(name="psA", bufs=2, space="PSUM") as psA:
            # --- P2: pre-conv over local window -> yla_sb ---
            loc_chunks = [(0, 4), (4, 4), (8, 4), (12, 4), (16, 4), (20, 4),
                          (24, 2)]
            for m, nr in loc_chunks:
                ps = psA.tile([C, 4 * W], f32, tag="conv_ps")
                conv6(ps, xl3, m, nr, wpre_sb, wpre2_sb, True)
                nc.scalar.activation(
                    out=yla_sb[0:C, m * W:(m + nr) * W],
                    in_=ps[:, :nr * W], func=RELU, bias=b1_sb[:, 0:1], scale=1.0,
                )
            # --- q projection (after P2) ---
            ioff = 0
            for NI in NI_SIZES:
                ps = psA.tile([128, 512], f32, tag="q_ps")
                nc.tensor.matmul(ps[:, :NI], wq4_sb[:], yla_sb[:, ioff:ioff + NI],
                                 start=True, stop=True)
                nc.vector.tensor_copy(out=q_sb[:, ioff:ioff + NI],
                                      in_=ps[:, :NI])
                ioff += NI

            # --- P1 pre-conv over full sample, with the k / vT projection
            # matmuls interleaved as their ya spans complete ---
            VB = C + 1  # 65

            def k_quad(quad):
                nch = 4 if quad < 4 else 2
                ps = psA.tile([128, 512], f32, tag="k_ps")
                for t in range(nch):
                    c4 = quad * 4 + t
                    nc.tensor.matmul(
                        ps[32 * t:32 * t + CQK, :],
                        wk_sb[:],
                        ya_sb[:, c4 * 512:(c4 + 1) * 512],
                        start=True, stop=True,
                        tile_position=(0, 32 * t),
                    )
                nc.vector.tensor_copy(
                    out=k_sb[:, quad * 512:(quad + 1) * 512], in_=ps[:])

            def vt_grp(grp):
                ps = psA.tile([128, 6 * C], f32, tag="vt_ps")
                for t in range(6):
                    jb = grp * 6 + t
                    nc.tensor.matmul(
                        ps[:, t * C:(t + 1) * C],
                        ya_sb[:, jb * JB:(jb + 1) * JB],
                        wv_sb[:, 0:C], start=True, stop=True,
                    )
                nc.vector.tensor_copy(
                    out=vt3[:, grp * 6:(grp + 1) * 6, 0:C],
                    in_=ps.rearrange("p (j c) -> p j c", c=C))

            kq_done = vt_done = 0
            for ch in range(H // 4):
                ps = psA.tile([C, 4 * W], f32, tag="conv_ps")
                conv6(ps, xf3, ch * 4, 4, wpre_sb, wpre2_sb, True)
                nc.scalar.activation(
                    out=ya_sb[0:C, ch * 4 * W:(ch + 1) * 4 * W],
                    in_=ps[:], func=RELU, bias=b1_sb[:, 0:1], scale=1.0,
                )
                avail = (ch + 1) * 4 * W   # ya columns complete
                while kq_done < 5 and 2048 * (kq_done + 1) <= avail:
                    k_quad(kq_done)
                    kq_done += 1
                while vt_done < 12 and 768 * (vt_done + 1) <= avail:
                    vt_grp(vt_done)
                    vt_done += 1
            while kq_done < 5:
                k_quad(kq_done)
                kq_done += 1
            while vt_done < 12:
                vt_grp(vt_done)
                vt_done += 1

        # --- P4: attention ---
        with tc.tile_pool(name="et_ps", bufs=2, space="PSUM") as et_ps, \
             tc.tile_pool(name="acc_ps", bufs=2, space="PSUM") as acc_ps, \
             tc.tile_pool(name="p_pool", bufs=5) as p_pool, \
             tc.tile_pool(name="dram", bufs=2, space="DRAM") as dpool, \
             tc.tile_pool(name="ep_pool", bufs=2) as ep_pool:
            NG = NJB // JG
            LAG = 3   # PV trails exp by 3 groups so the PE never waits on exp
            ioff = 0
            prev_full = 0
            for ci, NI in enumerate(NI_SIZES):
                acc = acc_ps.tile([VB, 512], f32, tag="acc")
                p_tiles = {}
                for g in range(NG + LAG):
                    if g < NG:
                        et = et_ps.tile([128, JG * 512], f32, tag="et")
                        p = p_pool.tile([128, JG * 512], bf16, tag="p")
                        p_tiles[g] = p
                        for t in range(JG):
                            jb = jbseq[g * JG + t]
                            c4 = jb // 4
                            s = c4 % 4
                            col = 512 * (c4 // 4) + 128 * (jb % 4)
                            nc.tensor.matmul(
                                et[:, t * 512:t * 512 + NI],
                                k_sb[32 * s:32 * s + CQK, col:col + JB],
                                q_sb[32 * s:32 * s + CQK, ioff:ioff + NI],
                                start=True, stop=True,
                                tile_position=(32 * s, 0),
                            )
                        # split each group's exp across BOTH engines so the
                        # et tile double-buffers (6 banks) and neither engine
                        # sits on the PE critical path.
                        HALF = JG * 256
                        nc.scalar.activation(out=p[:, 0:HALF],
                                             in_=et[:, 0:HALF], func=EXP)
                        nc.vector.tensor_scalar(
                            out=p[:, HALF:2 * HALF].bitcast(u16),
                            in0=et[:, HALF:2 * HALF],
                            scalar1=SEXP_A, scalar2=SEXP_B,
                            op0=mybir.AluOpType.mult,
                            op1=mybir.AluOpType.add,
                        )
                    gp = g - LAG
                    if gp < 0:
                        continue
                    p = p_tiles.pop(gp)
                    for t in range(JG):
                        jb = jbseq[gp * JG + t]
                        nc.tensor.matmul(
                            acc[:, :NI],
                            vt_sb[:, jb * VB:(jb + 1) * VB],
                            p[:, t * 512:t * 512 + NI],
                            start=(gp == 0 and t == 0),
                            stop=(gp == NG - 1 and t == JG - 1),
                        )
                # epilogue: of = acc[0:64] * (1/S) + y_loc, streamed into the
                # padded (and shift-1-copy) layout for the final conv.
                # evacuate acc to SBUF on the (less loaded) scalar engine so
                # the PSUM bank frees early and the DVE stays on exp.
                acc_sb = ep_pool.tile([VB, 512], f32, tag="accsb")
                nc.scalar.copy(out=acc_sb[:, :NI], in_=acc[:, :NI])
                # reciprocal is iterative per-element on DVE: reshape the
                # [1,NI] row to [NI/4,4] via a DRAM hop so it costs ~150ns.
                np4 = NI // 4
                srd = dpool.tile([1, 512], f32, tag="srd")
                nc.sync.dma_start(out=srd[:, :NI], in_=acc_sb[C:C + 1, :NI])
                sr = ep_pool.tile([128, 4], f32, tag="sr")
                nc.sync.dma_start(out=sr[:np4, :],
                                  in_=srd[:, :NI].rearrange("o (p f) -> (o p) f", f=4))
                rr = ep_pool.tile([128, 4], f32, tag="rr")
                nc.vector.reciprocal(rr[:np4, :], sr[:np4, :])
                rd = dpool.tile([1, 512], f32, tag="rd")
                nc.sync.dma_start(out=rd[:, :NI].rearrange("o (p f) -> (o p) f", f=4),
                                  in_=rr[:np4, :])
                rb_sb = ep_pool.tile([C, 512], f32, tag="rb")
                rd_bcast = bass.AP(tensor=rd.tensor, offset=rd.offset,
                                   ap=[[0, C]] + list(rd.ap[1:]))
                nc.sync.dma_start(out=rb_sb[:, :NI], in_=rd_bcast[:, :NI])
                of1 = ep_pool.tile([C, 512], f32, tag="of1")
                nc.vector.tensor_mul(of1[:, :NI], acc_sb[0:C, :NI], rb_sb[:, :NI])
                # residual add + scatter to ofp (both partition halves), on
                # gpsimd, in <=3 row-aligned pieces per half.
                pieces = []
                pos = ioff
                while pos < ioff + NI:
                    r0, cc = divmod(pos, W)
                    if cc == 0 and ioff + NI - pos >= W:
                        nr = (ioff + NI - pos) // W
                        pieces.append((pos, r0, 0, nr * W, nr))
                        pos += nr * W
                    else:
                        ln = min(W - cc, ioff + NI - pos)
                        pieces.append((pos, r0, cc, ln, 0))
                        pos += ln
                for (p0, r0, cc, ln, nr) in pieces:
                    o0 = p0 - ioff
                    if nr > 0:
                        nc.gpsimd.tensor_add(
                            ofp3[0:C, r0:r0 + nr, 1:1 + W],
                            of1[:, o0:o0 + ln].rearrange("p (r c) -> p r c", c=W),
                            yla_sb[0:C, p0:p0 + ln].rearrange("p (r c) -> p r c", c=W))
                    else:
                        nc.gpsimd.tensor_add(
                            ofp3[0:C, r0:r0 + 1, 1 + cc:1 + cc + ln],
                            of1[:, o0:o0 + ln],
                            yla_sb[0:C, p0:p0 + ln])
                # mask out-of-image halo rows once they are complete
                if ci == 0:
                    nc.gpsimd.tensor_mul(ofp3[0:C, 0:1, 1:1 + W],
                                         ofp3[0:C, 0:1, 1:1 + W],
                                         m2_sb[:, 0:W])
                if ci == len(NI_SIZES) - 1:
                    nc.gpsimd.tensor_mul(ofp3[0:C, LOCROWS - 1:LOCROWS, 1:1 + W],
                                         ofp3[0:C, LOCROWS - 1:LOCROWS, 1:1 + W],
                                         m2_sb[:, W:2 * W])
                # the shift-1 bottom half for the packed final conv: DMA can
                # cross partitions; copy fully-written top rows (post-mask).
                full = (ioff + NI) // W if ci < len(NI_SIZES) - 1 else LOCROWS
                lo = max(1, prev_full)
                if full > lo:
                    nc.sync.dma_start(
                        out=ofp3[C:2 * C, lo - 1:full - 1, :],
                        in_=ofp3[0:C, lo:full, :])
                prev_full = full
                ioff += NI

        # --- P5: final conv over own 24 rows (2-row-tap packed) ---
        with tc.tile_pool(name="psB", bufs=2, space="PSUM") as psB:
            for ch in range(ROWS // 4):
                ps = psB.tile([C, 4 * W], f32, tag="fin_ps")
                conv6(ps, ofp3, ch * 4, 4, wfin_sb, wfin2_sb, True)
                nc.scalar.activation(
                    out=out_sb[:, ch * 4 * W:(ch + 1) * 4 * W],
                    in_=ps[:], func=RELU, bias=b2_sb[:, 0:1], scale=1.0,
                )
                nc.sync.dma_start(out=out_d[:, ch * 4 * W:(ch + 1) * 4 * W],
                                  in_=out_sb[:, ch * 4 * W:(ch + 1) * 4 * W])

    if split_waits:
        _split_excess_waits(nc)
    _NC_CACHE[key] = nc
    return nc


# ---------------------------------------------------------------------------
# host-side prep + launch
# ---------------------------------------------------------------------------

def _prep_in_maps(x, w_pre, bn1_g, bn1_b, bn1_m, bn1_v, wq, bq, wk, bk, wv, bv,
                  w_fin, bn2_g, bn2_b, bn2_m, bn2_v, gamma):
    x = np.asarray(x, np.float32)
    inv1 = 1.0 / np.sqrt(np.asarray(bn1_v, np.float32) + EPS)
    s1 = np.asarray(bn1_g, np.float32) * inv1
    wpre_f = np.asarray(w_pre, np.float32) * s1[:, None, None, None]
    b1f = np.asarray(bn1_b, np.float32) - np.asarray(bn1_m, np.float32) * s1
    inv2 = 1.0 / np.sqrt(np.asarray(bn2_v, np.float32) + EPS)
    s2 = np.asarray(bn2_g, np.float32) * inv2
    wfin_f = np.asarray(w_fin, np.float32) * s2[:, None, None, None]
    b2f = np.asarray(bn2_b, np.float32) - np.asarray(bn2_m, np.float32) * s2
    gma = float(np.asarray(gamma, np.float32).reshape(-1)[0])

    def pack2(wf):
        # 2-row-packed conv weights: [dr0|dr1] on 128 partitions, dr2 alone
        wt = wf.transpose(1, 2, 3, 0)        # [cin, dr, ds, cout]
        wpk = np.concatenate([wt[:, 0], wt[:, 1]], axis=0)  # [128, 3, 64]
        return (wpk.reshape(2 * C, 3 * C).astype(BF16),
                wt[:, 2].reshape(C, 3 * C).astype(BF16))

    wpre_pack, wpre2 = pack2(wpre_f)
    wfin_pack, wfin2 = pack2(wfin_f)

    wq2 = np.asarray(wq, np.float32).reshape(CQK, C)
    wk2 = np.asarray(wk, np.float32).reshape(CQK, C)
    wv2 = np.asarray(wv, np.float32).reshape(C, C)
    wq_aug = np.concatenate([wq2.T, np.asarray(bq, np.float32)[None, :]], 0)
    wq4 = np.zeros((C + 1, 128), np.float32)
    for s in range(4):
        wq4[:, 32 * s:32 * s + CQK] = wq_aug
    wq4 = wq4.astype(BF16)
    wk_aug = np.concatenate([wk2.T, np.asarray(bk, np.float32)[None, :]], 0).astype(BF16)
    # gamma folded into v (the ones column stays unscaled so the softmax
    # denominator is exact)
    wv_aug = np.zeros((C + 1, C + 1), np.float32)
    wv_aug[0:C, 0:C] = wv2.T * gma
    wv_aug[C, 0:C] = np.asarray(bv, np.float32) * gma
    wv_aug[C, C] = 1.0
    wv_aug = wv_aug.astype(BF16)

    b1f = b1f.reshape(C, 1)
    b2f = b2f.reshape(C, 1)
    ones = np.ones((1, N), BF16)

    xpad = np.zeros((B, C, HP, WP), np.float32)
    xpad[:, :, 1:1 + H, 1:1 + W] = x
    xpad_bf = xpad.astype(BF16)

    in_maps = []
    for core in range(8):
        b, qc = divmod(core, QCH)
        xf = xpad_bf[b].reshape(C, HP * WP)
        # local window: image rows [24q-2, 24q+26) = padded rows [24q-1, 24q+27)
        xl = np.zeros((C, LOCP, WP), np.float32)
        pr0 = ROWS * qc - 1
        lo = max(0, -pr0)
        hi = min(LOCP, HP - pr0)
        xl[:, lo:hi, :] = xpad[b, :, pr0 + lo:pr0 + hi, :]
        xl = xl.reshape(C, LOCP * WP).astype(BF16)
        m2 = np.ones((C, 2 * W), np.float32)
        if qc == 0:
            m2[:, 0:W] = 0.0
        if qc == QCH - 1:
            m2[:, W:2 * W] = 0.0
        in_maps.append({
            "xf": xf, "xl": xl, "wpre": wpre_pack, "wpre2": wpre2, "b1": b1f,
            "wfin": wfin_pack, "wfin2": wfin2, "b2": b2f, "wq4": wq4,
            "wk": wk_aug, "wv": wv_aug, "ones": ones, "m2": m2,
        })
    return in_maps


def kernel(**inputs):
    from concourse.bass_utils import run_bass_kernel_spmd

    nc = _build_nc()
    in_maps = _prep_in_maps(**inputs)
    res = run_bass_kernel_spmd(nc, in_maps, list(range(8)))
    out = np.zeros((B, C, H, W), np.float32)
    for core in range(8):
        b, qc = divmod(core, QCH)
        out[b, :, ROWS * qc:ROWS * (qc + 1), :] = \
            res.results[core]["out"].reshape(C, ROWS, W)
    return out


# revision 39
# speedup vs baseline: 1.0485x; 1.0485x over previous
"""Trainium2 Bass kernel for nn_AttentionLayer (pre-conv + self-attention + final conv).

Sharding: 8 cores = 2 samples x 4 query-row chunks. Each core computes the
full pre-conv y for its sample (k/v need all N=9216 positions), attention for
its 26-row query window (24 own rows + 1 halo row each side for the final
3x3 conv), and the final conv for its 24 output rows.

Perf structure (v2):
 - exp is split between ScalarE (native Exp) and VectorE (Schraudolph int16
   trick: bf16 bits = round(x * 2^7/ln2 + (127*2^7 - 5.51)), written as
   uint16 and bitcast to bf16) in alternating groups of 3 j-blocks, so both
   engines chew the 23M-element softmax concurrently.
 - energy matmuls run 4-way concurrent via tile_position row groups; k is
   laid out in 4 partition stripes directly by col-tiled projection matmuls,
   q is replicated to all 4 stripes by a single 4-replica weight matrix.
 - softmax denominator comes from an augmented ones-column in vT; the
   divide is reciprocal_approx_fast on [1,NI] + gpsimd partition_broadcast;
   the residual-add + padded-layout scatter runs on gpsimd.
 - the final 3x3 conv is 2-row-tap packed (6 matmuls per 4 rows).
"""

import os
import hashlib
import shutil

import numpy as np
import ml_dtypes

BF16 = ml_dtypes.bfloat16
EPS = 1e-5

B, C, CQK, H, W = 2, 64, 16, 96, 96
N = H * W                       # 9216
QCH = 4                         # query chunks per sample
ROWS = H // QCH                 # 24 rows per core
LOCROWS = ROWS + 2              # 26 (with halo)
NLOC = LOCROWS * W              # 2496
HP, WP = H + 2, W + 2           # 98x98 padded frame
LOCP = LOCROWS + 2              # 28 padded local rows
NI_SIZES = [512, 512, 512, 512, 448]   # i-chunks over NLOC
JB = 128                        # j-block height
NJB = N // JB                   # 72
JG = 3                          # j-blocks per exp group
# Schraudolph bf16-bits exp constants (calibrated on the real energy range
# [-5.2, 4.5]; max rel err ~3.3%)
SEXP_A = 128.0 / float(np.log(2.0))
SEXP_B = 127.0 * 128.0 - 5.51


def _jb_sequence():
    """j-block processing order: rotate the 4 k-stripes every matmul."""
    seq = []
    for u in range(4):          # chunk quads 0..15
        for b in range(4):
            for s in range(4):
                seq.append(4 * (4 * u + s) + b)
    for b in range(4):          # chunks 16,17 (stripes 0,1)
        for s in range(2):
            seq.append(4 * (16 + s) + b)
    assert sorted(seq) == list(range(NJB))
    return seq


# ---------------------------------------------------------------------------
# framework patches (self-contained)
# ---------------------------------------------------------------------------

def _apply_patches():
    import concourse.tile as tile
    import concourse.bass_utils as bu
    import concourse.bass2jax as b2j
    from concourse import mybir

    # 1) walrus in this env rejects >1-2 sync waits on the final Drain
    #    (CTRL_NO_STRUCT): split waits into single-wait nops.
    def _drain_and_barrier_split(self, tick_clock, wait_clock):
        nc = self.nc
        probe = nc.sync.nop()
        wait_clock.add_sem_waits(
            probe.ins, tile.ScopedClock({None: tick_clock.global_clock})
        )
        waits = list(probe.ins.sync_info.on_wait) if probe.ins.sync_info else []
        if probe.ins.sync_info is not None:
            probe.ins.sync_info.on_wait = []
        for w in waits[:-1]:
            nop = nc.sync.nop()
            if nop.ins.sync_info is None:
                nop.ins.sync_info = mybir.SyncInfo(on_wait=[w], on_update=[])
            else:
                nop.ins.sync_info.on_wait.append(w)
        drain_inst = nc.sync.drain()
        if waits:
            if drain_inst.ins.sync_info is None:
                drain_inst.ins.sync_info = mybir.SyncInfo(
                    on_wait=[waits[-1]], on_update=[]
                )
            else:
                drain_inst.ins.sync_info.on_wait.append(waits[-1])
        nc.all_engine_barrier()
        assert self.sems is not None
        popped = nc._tile_sem_poison_stack.pop()
        assert popped is self._sem_poison
        nc.clear_and_free_semaphores(list(self.sems.allocated().values()))
        nc.all_engine_barrier()

    tile.TileContext._drain_and_barrier = _drain_and_barrier_split

    # 2) NEFF disk cache keyed by BIR hash (compile is deterministic).
    cache_dir = os.path.join(os.path.dirname(os.path.abspath(__file__)),
                             ".neff_cache")
    try:
        os.makedirs(cache_dir, exist_ok=True)
    except OSError:
        cache_dir = None
    _orig_compile = bu.compile_bir_kernel

    def cached_compile(bir_json, tmpdir, neff_name="file.neff"):
        if cache_dir is None:
            return _orig_compile(bir_json, tmpdir, neff_name)
        h = hashlib.sha256(bir_json).hexdigest()[:24]
        cpath = os.path.join(cache_dir, f"{h}.neff")
        out = os.path.join(tmpdir, neff_name)
        if os.path.exists(cpath):
            shutil.copyfile(cpath, out)
            return out
        r = _orig_compile(bir_json, tmpdir, neff_name)
        try:
            shutil.copyfile(r, cpath)
        except OSError:
            pass
        return r

    bu.compile_bir_kernel = cached_compile
    b2j.compile_bir_kernel = cached_compile


def _split_excess_waits(nc, max_waits=1):
    """walrus in this env allows only a couple of sync-wait slots per
    instruction; move excess waits onto preceding same-engine NOPs."""
    from concourse import mybir
    idx = 0
    for f in nc.m.functions:
        for bb in f.blocks:
            new = []
            changed = False
            for inst in bb.instructions:
                si = inst.sync_info
                waits = list(si.on_wait) if si is not None and si.on_wait else []
                if len(waits) > max_waits:
                    changed = True
                    for w in waits[:-max_waits]:
                        idx += 1
                        nop = mybir.InstNoOp(name=f"wsplit_{idx}", ins=[], outs=[])
                        nop.engine = inst.engine
                        nop.sync_info = mybir.SyncInfo(on_wait=[w], on_update=[])
                        new.append(nop)
                    si.on_wait = waits[-max_waits:]
                new.append(inst)
            if changed:
                bb.instructions = new


# ---------------------------------------------------------------------------
# device program
# ---------------------------------------------------------------------------

_NC_CACHE = {}


def _build_nc(split_waits=True):
    key = ("nc", split_waits)
    if key in _NC_CACHE:
        return _NC_CACHE[key]
    _apply_patches()
    import concourse.bass as bass
    import concourse.tile as tile
    from concourse import mybir
    from contextlib import ExitStack

    f32 = mybir.dt.float32
    bf16 = mybir.dt.bfloat16
    u16 = mybir.dt.uint16
    RELU = mybir.ActivationFunctionType.Relu
    EXP = mybir.ActivationFunctionType.Exp

    nc = bass.Bass()

    xf_d = nc.declare_dram_parameter("xf", [C, HP * WP], bf16, isOutput=False)
    xl_d = nc.declare_dram_parameter("xl", [C, LOCP * WP], bf16, isOutput=False)
    # pre/final conv weights: taps (dr0|dr1) stacked on 128 partitions, dr2 sep
    wpre_d = nc.declare_dram_parameter("wpre", [2 * C, 3 * C], bf16, isOutput=False)
    wpre2_d = nc.declare_dram_parameter("wpre2", [C, 3 * C], bf16, isOutput=False)
    b1_d = nc.declare_dram_parameter("b1", [C, 1], f32, isOutput=False)
    wfin_d = nc.declare_dram_parameter("wfin", [2 * C, 3 * C], bf16, isOutput=False)
    wfin2_d = nc.declare_dram_parameter("wfin2", [C, 3 * C], bf16, isOutput=False)
    b2_d = nc.declare_dram_parameter("b2", [C, 1], f32, isOutput=False)
    wq4_d = nc.declare_dram_parameter("wq4", [C + 1, 128], bf16, isOutput=False)
    wk_d = nc.declare_dram_parameter("wk", [C + 1, CQK], bf16, isOutput=False)
    wv_d = nc.declare_dram_parameter("wv", [C + 1, C + 1], bf16, isOutput=False)
    ones_d = nc.declare_dram_parameter("ones", [1, N], bf16, isOutput=False)
    m2_d = nc.declare_dram_parameter("m2", [C, 2 * W], f32, isOutput=False)
    out_d = nc.declare_dram_parameter("out", [C, ROWS * W], f32, isOutput=True)

    jbseq = _jb_sequence()

    with tile.TileContext(nc) as tc, ExitStack() as ctx:
        consts = ctx.enter_context(tc.tile_pool(name="consts", bufs=1))
        bigs = ctx.enter_context(tc.tile_pool(name="bigs", bufs=1))

        # --- constants ---
        wpre_sb = consts.tile([2 * C, 3 * C], bf16)
        wpre2_sb = consts.tile([C, 3 * C], bf16)
        wfin_sb = consts.tile([2 * C, 3 * C], bf16)
        wfin2_sb = consts.tile([C, 3 * C], bf16)
        b1_sb = consts.tile([C, 1], f32)
        b2_sb = consts.tile([C, 1], f32)
        wq4_sb = consts.tile([C + 1, 128], bf16)
        wk_sb = consts.tile([C + 1, CQK], bf16)
        wv_sb = consts.tile([C + 1, C + 1], bf16)
        m2_sb = consts.tile([C, 2 * W], f32)
        junk_sb = consts.tile([C, 1], f32)
        nc.sync.dma_start(out=b1_sb, in_=b1_d[:])
        nc.sync.dma_start(out=wpre_sb, in_=wpre_d[:])
        nc.sync.dma_start(out=wpre2_sb, in_=wpre2_d[:])
        # preload the exp table set before the first ReLU so the kernel pays
        # exactly one ACT table load, during the startup DMA window.
        nc.scalar.activation(out=junk_sb, in_=b1_sb, func=EXP)

        # --- x frames, rows dr0 on partitions 0-63 / dr1 (shift 1 row) 64-127
        xf_sb = bigs.tile([2 * C, HP * WP], bf16)
        xl_sb = bigs.tile([2 * C, LOCP * WP], bf16)
        nc.sync.dma_start(out=xl_sb[0:C, :], in_=xl_d[:])
        nc.sync.dma_start(out=xl_sb[C:2 * C, 0:(LOCP - 1) * WP],
                          in_=xl_d[:, WP:LOCP * WP])
        for r0, r1 in [(0, 26), (26, 50), (50, 74), (74, HP)]:
            nc.sync.dma_start(out=xf_sb[0:C, r0 * WP:r1 * WP],
                              in_=xf_d[:, r0 * WP:r1 * WP])
        for r0, r1 in [(0, 50), (50, HP - 1)]:
            nc.sync.dma_start(out=xf_sb[C:2 * C, r0 * WP:r1 * WP],
                              in_=xf_d[:, (r0 + 1) * WP:(r1 + 1) * WP])
        nc.sync.dma_start(out=wfin_sb, in_=wfin_d[:])
        nc.sync.dma_start(out=wfin2_sb, in_=wfin2_d[:])
        nc.sync.dma_start(out=b2_sb, in_=b2_d[:])
        nc.sync.dma_start(out=wq4_sb, in_=wq4_d[:])
        nc.sync.dma_start(out=wk_sb, in_=wk_d[:])
        nc.sync.dma_start(out=wv_sb, in_=wv_d[:])
        nc.sync.dma_start(out=m2_sb, in_=m2_d[:])

        xf3 = xf_sb.rearrange("p (r c) -> p r c", c=WP)
        xl3 = xl_sb.rearrange("p (r c) -> p r c", c=WP)

        ya_sb = bigs.tile([C + 1, N], bf16)       # y_aug (full sample)
        yla_sb = bigs.tile([C + 1, NLOC], bf16)   # y_aug (local window)
        k_sb = bigs.tile([128, 2560], bf16)       # k in 4 partition stripes
        q_sb = bigs.tile([128, NLOC], bf16)       # q replicated at 4 stripes
        vt_sb = bigs.tile([128, NJB * (C + 1)], bf16)
        ofp_sb = bigs.tile([2 * C, LOCROWS * WP], bf16)  # padded out_feat,
        #   partitions 64-127 = shift-1-row copy for the packed final conv
        out_sb = bigs.tile([C, ROWS * W], f32)

        nc.sync.dma_start(out=ya_sb[C:C + 1, :], in_=ones_d[:])
        nc.sync.dma_start(out=yla_sb[C:C + 1, :], in_=ones_d[:, 0:NLOC])
        # ones column of vT (softmax denominator maker)
        vt3 = vt_sb.rearrange("p (j c) -> p j c", c=C + 1)
        nc.vector.memset(vt3[:, :, C:C + 1], 1.0)
        # zero the pad columns of the padded out_feat layout
        ofp3 = ofp_sb.rearrange("p (r c) -> p r c", c=WP)
        nc.vector.memset(ofp3[:, :, 0:1], 0.0)
        nc.vector.memset(ofp3[:, :, WP - 1:WP], 0.0)

        def conv6(ps, x3, r, nr, wp_sb, wp2_sb, stop_dr2):
            """6-matmul 3x3 conv chunk: rows r..r+nr of the padded frame."""
            for ds in range(3):
                nc.tensor.matmul(
                    ps[:, :nr * W],
                    wp_sb[:, ds * C:(ds + 1) * C],
                    x3[:, r:r + nr, ds:ds + W],
                    start=(ds == 0), stop=False,
                )
            for ds in range(3):
                nc.tensor.matmul(
                    ps[:, :nr * W],
                    wp2_sb[:, ds * C:(ds + 1) * C],
                    x3[0:C, r + 2:r + 2 + nr, ds:ds + W],
                    start=False, stop=(stop_dr2 and ds == 2),
                )

        with tc.tile_pool(name="psA", bufs=2, space="PSUM") as psA:
            # --- P2: pre-conv over local window -> yla_sb ---
            loc_chunks = [(0, 4), (4, 4), (8, 4), (12, 4), (16, 4), (20, 4),
                          (24, 2)]
            for m, nr in loc_chunks:
                ps = psA.tile([C, 4 * W], f32, tag="conv_ps")
                conv6(ps, xl3, m, nr, wpre_sb, wpre2_sb, True)
                nc.scalar.activation(
                    out=yla_sb[0:C, m * W:(m + nr) * W],
                    in_=ps[:, :nr * W], func=RELU, bias=b1_sb[:, 0:1], scale=1.0,
                )
            # --- q projection (after P2) ---
            ioff = 0
            for NI in NI_SIZES:
                ps = psA.tile([128, 512], f32, tag="q_ps")
                nc.tensor.matmul(ps[:, :NI], wq4_sb[:], yla_sb[:, ioff:ioff + NI],
                                 start=True, stop=True)
                nc.vector.tensor_copy(out=q_sb[:, ioff:ioff + NI],
                                      in_=ps[:, :NI])
                ioff += NI

            # --- P1 pre-conv over full sample, with the k / vT projection
            # matmuls interleaved as their ya spans complete ---
            VB = C + 1  # 65

            def k_quad(quad):
                nch = 4 if quad < 4 else 2
                ps = psA.tile([128, 512], f32, tag="k_ps")
                for t in range(nch):
                    c4 = quad * 4 + t
                    nc.tensor.matmul(
                        ps[32 * t:32 * t + CQK, :],
                        wk_sb[:],
                        ya_sb[:, c4 * 512:(c4 + 1) * 512],
                        start=True, stop=True,
                        tile_position=(0, 32 * t),
                    )
                nc.vector.tensor_copy(
                    out=k_sb[:, quad * 512:(quad + 1) * 512], in_=ps[:])

            def vt_grp(grp):
                ps = psA.tile([128, 6 * C], f32, tag="vt_ps")
                for t in range(6):
                    jb = grp * 6 + t
                    nc.tensor.matmul(
                        ps[:, t * C:(t + 1) * C],
                        ya_sb[:, jb * JB:(jb + 1) * JB],
                        wv_sb[:, 0:C], start=True, stop=True,
                    )
                nc.vector.tensor_copy(
                    out=vt3[:, grp * 6:(grp + 1) * 6, 0:C],
                    in_=ps.rearrange("p (j c) -> p j c", c=C))

            kq_done = vt_done = 0
            for ch in range(H // 4):
                ps = psA.tile([C, 4 * W], f32, tag="conv_ps")
                conv6(ps, xf3, ch * 4, 4, wpre_sb, wpre2_sb, True)
                nc.scalar.activation(
                    out=ya_sb[0:C, ch * 4 * W:(ch + 1) * 4 * W],
                    in_=ps[:], func=RELU, bias=b1_sb[:, 0:1], scale=1.0,
                )
                avail = (ch + 1) * 4 * W   # ya columns complete
                while kq_done < 5 and 2048 * (kq_done + 1) <= avail:
                    k_quad(kq_done)
                    kq_done += 1
                while vt_done < 12 and 768 * (vt_done + 1) <= avail:
                    vt_grp(vt_done)
                    vt_done += 1
            while kq_done < 5:
                k_quad(kq_done)
                kq_done += 1
            while vt_done < 12:
                vt_grp(vt_done)
                vt_done += 1

        # --- P4: attention ---
        with tc.tile_pool(name="etA_ps", bufs=1, space="PSUM") as etA_ps, \
             tc.tile_pool(name="etD_ps", bufs=1, space="PSUM") as etD_ps, \
             tc.tile_pool(name="acc_ps", bufs=2, space="PSUM") as acc_ps, \
             tc.tile_pool(name="pA_pool", bufs=4) as pA_pool, \
             tc.tile_pool(name="pD_pool", bufs=4) as pD_pool, \
             tc.tile_pool(name="dram", bufs=2, space="DRAM") as dpool, \
             tc.tile_pool(name="ep_pool", bufs=2) as ep_pool:
            NG = NJB // JG
            LAG = 3   # PV trails exp by 3 groups so the PE never waits on exp
            ioff = 0
            prev_full = 0
            for ci, NI in enumerate(NI_SIZES):
                acc = acc_ps.tile([VB, 512], f32, tag="acc")
                p_tiles = {}
                for g in range(NG + LAG):
                    if g < NG:
                        use_act = (g % 2 == 0)
                        if use_act:
                            et = etA_ps.tile([128, JG * 512], f32, tag="etA")
                            p = pA_pool.tile([128, JG * 512], bf16, tag="pA")
                        else:
                            et = etD_ps.tile([128, JG * 512], f32, tag="etD")
                            p = pD_pool.tile([128, JG * 512], bf16, tag="pD")
                        p_tiles[g] = p
                        for t in range(JG):
                            jb = jbseq[g * JG + t]
                            c4 = jb // 4
                            s = c4 % 4
                            col = 512 * (c4 // 4) + 128 * (jb % 4)
                            nc.tensor.matmul(
                                et[:, t * 512:t * 512 + NI],
                                k_sb[32 * s:32 * s + CQK, col:col + JB],
                                q_sb[32 * s:32 * s + CQK, ioff:ioff + NI],
                                start=True, stop=True,
                                tile_position=(32 * s, 0),
                            )
                        if use_act:
                            nc.scalar.activation(out=p[:], in_=et[:], func=EXP)
                        else:
                            nc.vector.tensor_scalar(
                                out=p[:].bitcast(u16), in0=et[:],
                                scalar1=SEXP_A, scalar2=SEXP_B,
                                op0=mybir.AluOpType.mult,
                                op1=mybir.AluOpType.add,
                            )
                    gp = g - LAG
                    if gp < 0:
                        continue
                    p = p_tiles.pop(gp)
                    for t in range(JG):
                        jb = jbseq[gp * JG + t]
                        nc.tensor.matmul(
                            acc[:, :NI],
                            vt_sb[:, jb * VB:(jb + 1) * VB],
                            p[:, t * 512:t * 512 + NI],
                            start=(gp == 0 and t == 0),
                            stop=(gp == NG - 1 and t == JG - 1),
                        )
                # epilogue: of = acc[0:64] * (1/S) + y_loc, streamed into the
                # padded (and shift-1-copy) layout for the final conv.
                # evacuate acc to SBUF on the (less loaded) scalar engine so
                # the PSUM bank frees early and the DVE stays on exp.
                acc_sb = ep_pool.tile([VB, 512], f32, tag="accsb")
                nc.scalar.copy(out=acc_sb[:, :NI], in_=acc[:, :NI])
                # reciprocal is iterative per-element on DVE: reshape the
                # [1,NI] row to [NI/4,4] via a DRAM hop so it costs ~150ns.
                np4 = NI // 4
                srd = dpool.tile([1, 512], f32, tag="srd")
                nc.sync.dma_start(out=srd[:, :NI], in_=acc_sb[C:C + 1, :NI])
                sr = ep_pool.tile([128, 4], f32, tag="sr")
                nc.sync.dma_start(out=sr[:np4, :],
                                  in_=srd[:, :NI].rearrange("o (p f) -> (o p) f", f=4))
                rr = ep_pool.tile([128, 4], f32, tag="rr")
                nc.vector.reciprocal(rr[:np4, :], sr[:np4, :])
                rd = dpool.tile([1, 512], f32, tag="rd")
                nc.sync.dma_start(out=rd[:, :NI].rearrange("o (p f) -> (o p) f", f=4),
                                  in_=rr[:np4, :])
                rb_sb = ep_pool.tile([C, 512], f32, tag="rb")
                rd_bcast = bass.AP(tensor=rd.tensor, offset=rd.offset,
                                   ap=[[0, C]] + list(rd.ap[1:]))
                nc.sync.dma_start(out=rb_sb[:, :NI], in_=rd_bcast[:, :NI])
                of1 = ep_pool.tile([C, 512], f32, tag="of1")
                nc.vector.tensor_mul(of1[:, :NI], acc_sb[0:C, :NI], rb_sb[:, :NI])
                # residual add + scatter to ofp (both partition halves), on
                # gpsimd, in <=3 row-aligned pieces per half.
                pieces = []
                pos = ioff
                while pos < ioff + NI:
                    r0, cc = divmod(pos, W)
                    if cc == 0 and ioff + NI - pos >= W:
                        nr = (ioff + NI - pos) // W
                        pieces.append((pos, r0, 0, nr * W, nr))
                        pos += nr * W
                    else:
                        ln = min(W - cc, ioff + NI - pos)
                        pieces.append((pos, r0, cc, ln, 0))
                        pos += ln
                for (p0, r0, cc, ln, nr) in pieces:
                    o0 = p0 - ioff
                    if nr > 0:
                        nc.gpsimd.tensor_add(
                            ofp3[0:C, r0:r0 + nr, 1:1 + W],
                            of1[:, o0:o0 + ln].rearrange("p (r c) -> p r c", c=W),
                            yla_sb[0:C, p0:p0 + ln].rearrange("p (r c) -> p r c", c=W))
                    else:
                        nc.gpsimd.tensor_add(
                            ofp3[0:C, r0:r0 + 1, 1 + cc:1 + cc + ln],
                            of1[:, o0:o0 + ln],
                            yla_sb[0:C, p0:p0 + ln])
                # mask out-of-image halo rows once they are complete
                if ci == 0:
                    nc.gpsimd.tensor_mul(ofp3[0:C, 0:1, 1:1 + W],
                                         ofp3[0:C, 0:1, 1:1 + W],
                                         m2_sb[:, 0:W])
                if ci == len(NI_SIZES) - 1:
                    nc.gpsimd.tensor_mul(ofp3[0:C, LOCROWS - 1:LOCROWS, 1:1 + W],
                                         ofp3[0:C, LOCROWS - 1:LOCROWS, 1:1 + W],
                                         m2_sb[:, W:2 * W])
                # the shift-1 bottom half for the packed final conv: DMA can
                # cross partitions; copy fully-written top rows (post-mask).
                full = (ioff + NI) // W if ci < len(NI_SIZES) - 1 else LOCROWS
                lo = max(1, prev_full)
                if full > lo:
                    nc.sync.dma_start(
                        out=ofp3[C:2 * C, lo - 1:full - 1, :],
                        in_=ofp3[0:C, lo:full, :])
                prev_full = full
                ioff += NI

        # --- P5: final conv over own 24 rows (2-row-tap packed) ---
        with tc.tile_pool(name="psB", bufs=2, space="PSUM") as psB:
            for ch in range(ROWS // 4):
                ps = psB.tile([C, 4 * W], f32, tag="fin_ps")
                conv6(ps, ofp3, ch * 4, 4, wfin_sb, wfin2_sb, True)
                nc.scalar.activation(
                    out=out_sb[:, ch * 4 * W:(ch + 1) * 4 * W],
                    in_=ps[:], func=RELU, bias=b2_sb[:, 0:1], scale=1.0,
                )
                nc.sync.dma_start(out=out_d[:, ch * 4 * W:(ch + 1) * 4 * W],
                                  in_=out_sb[:, ch * 4 * W:(ch + 1) * 4 * W])

    if split_waits:
        _split_excess_waits(nc)
    _NC_CACHE[key] = nc
    return nc


# ---------------------------------------------------------------------------
# host-side prep + launch
# ---------------------------------------------------------------------------

def _prep_in_maps(x, w_pre, bn1_g, bn1_b, bn1_m, bn1_v, wq, bq, wk, bk, wv, bv,
                  w_fin, bn2_g, bn2_b, bn2_m, bn2_v, gamma):
    x = np.asarray(x, np.float32)
    inv1 = 1.0 / np.sqrt(np.asarray(bn1_v, np.float32) + EPS)
    s1 = np.asarray(bn1_g, np.float32) * inv1
    wpre_f = np.asarray(w_pre, np.float32) * s1[:, None, None, None]
    b1f = np.asarray(bn1_b, np.float32) - np.asarray(bn1_m, np.float32) * s1
    inv2 = 1.0 / np.sqrt(np.asarray(bn2_v, np.float32) + EPS)
    s2 = np.asarray(bn2_g, np.float32) * inv2
    wfin_f = np.asarray(w_fin, np.float32) * s2[:, None, None, None]
    b2f = np.asarray(bn2_b, np.float32) - np.asarray(bn2_m, np.float32) * s2
    gma = float(np.asarray(gamma, np.float32).reshape(-1)[0])

    def pack2(wf):
        # 2-row-packed conv weights: [dr0|dr1] on 128 partitions, dr2 alone
        wt = wf.transpose(1, 2, 3, 0)        # [cin, dr, ds, cout]
        wpk = np.concatenate([wt[:, 0], wt[:, 1]], axis=0)  # [128, 3, 64]
        return (wpk.reshape(2 * C, 3 * C).astype(BF16),
                wt[:, 2].reshape(C, 3 * C).astype(BF16))

    wpre_pack, wpre2 = pack2(wpre_f)
    wfin_pack, wfin2 = pack2(wfin_f)

    wq2 = np.asarray(wq, np.float32).reshape(CQK, C)
    wk2 = np.asarray(wk, np.float32).reshape(CQK, C)
    wv2 = np.asarray(wv, np.float32).reshape(C, C)
    wq_aug = np.concatenate([wq2.T, np.asarray(bq, np.float32)[None, :]], 0)
    wq4 = np.zeros((C + 1, 128), np.float32)
    for s in range(4):
        wq4[:, 32 * s:32 * s + CQK] = wq_aug
    wq4 = wq4.astype(BF16)
    wk_aug = np.concatenate([wk2.T, np.asarray(bk, np.float32)[None, :]], 0).astype(BF16)
    # gamma folded into v (the ones column stays unscaled so the softmax
    # denominator is exact)
    wv_aug = np.zeros((C + 1, C + 1), np.float32)
    wv_aug[0:C, 0:C] = wv2.T * gma
    wv_aug[C, 0:C] = np.asarray(bv, np.float32) * gma
    wv_aug[C, C] = 1.0
    wv_aug = wv_aug.astype(BF16)

    b1f = b1f.reshape(C, 1)
    b2f = b2f.reshape(C, 1)
    ones = np.ones((1, N), BF16)

    xpad = np.zeros((B, C, HP, WP), np.float32)
    xpad[:, :, 1:1 + H, 1:1 + W] = x
    xpad_bf = xpad.astype(BF16)

    in_maps = []
    for core in range(8):
        b, qc = divmod(core, QCH)
        xf = xpad_bf[b].reshape(C, HP * WP)
        # local window: image rows [24q-2, 24q+26) = padded rows [24q-1, 24q+27)
        xl = np.zeros((C, LOCP, WP), np.float32)
        pr0 = ROWS * qc - 1
        lo = max(0, -pr0)
        hi = min(LOCP, HP - pr0)
        xl[:, lo:hi, :] = xpad[b, :, pr0 + lo:pr0 + hi, :]
        xl = xl.reshape(C, LOCP * WP).astype(BF16)
        m2 = np.ones((C, 2 * W), np.float32)
        if qc == 0:
            m2[:, 0:W] = 0.0
        if qc == QCH - 1:
            m2[:, W:2 * W] = 0.0
        in_maps.append({
            "xf": xf, "xl": xl, "wpre": wpre_pack, "wpre2": wpre2, "b1": b1f,
            "wfin": wfin_pack, "wfin2": wfin2, "b2": b2f, "wq4": wq4,
            "wk": wk_aug, "wv": wv_aug, "ones": ones, "m2": m2,
        })
    return in_maps


def kernel(**inputs):
    from concourse.bass_utils import run_bass_kernel_spmd

    nc = _build_nc()
    in_maps = _prep_in_maps(**inputs)
    res = run_bass_kernel_spmd(nc, in_maps, list(range(8)))
    out = np.zeros((B, C, H, W), np.float32)
    for core in range(8):
        b, qc = divmod(core, QCH)
        out[b, :, ROWS * qc:ROWS * (qc + 1), :] = \
            res.results[core]["out"].reshape(C, ROWS, W)
    return out
